# revision 20
# baseline (speedup 1.0000x reference)
"""Trainium2 Bass kernel for a top-2 MoE layer (B=2, S=2048, H=1024, F=4096, E=8).

Strategy: expert-parallel across 8 NeuronCores. The (tiny) gate runs on host in
numpy; each core runs one expert's fc1->relu->fc2 over only the tokens routed to
it, padded to a fixed capacity. The first C_main tokens (largest combine
weights) compute in float32r (full-rate PE matmul, ~1e-4 error); the tail
(capacity remainder, smallest combine weights) computes in bf16 — bf16 streams
at 1 cyc/col at any width, so the tail can be 128-granular instead of 256.
Host scatter-adds the combine-weighted expert outputs into the dense output.

Self-contained: hardcodes all shapes; imports only concourse + numpy.
"""

import numpy as np
import ml_dtypes
from contextlib import ExitStack

import concourse.bass as bass  # noqa: F401
import concourse.tile as tile
from concourse import bacc, mybir
from concourse.bass_utils import run_bass_kernel_spmd

B, S, H, F, E, K = 2, 2048, 1024, 4096, 8, 2
T = B * S
NH = H // 128   # 8 partition blocks of H
NF = F // 128   # 32 partition blocks of F
NFH = NF // 2   # f-blocks per F-half
F32 = mybir.dt.float32
F32R = mybir.dt.float32r
BF16 = mybir.dt.bfloat16
BF16_NP = ml_dtypes.bfloat16

_PROG_CACHE: dict = {}


def _build_program(CM, TC):
    """One expert's MLP over CM f32r tokens + TC bf16 tail tokens. Weights are
    streamed in a single pass by splitting F into two halves; the two halves'
    fc2 outputs go to separate DRAM tensors (host adds them)."""
    C = CM + TC
    nc = bacc.Bacc("TRN2", target_bir_lowering=False, debug=False, num_devices=E)

    xT = nc.dram_tensor("xT", [128, NH, CM], F32R, kind="ExternalInput").ap()
    w1d = nc.dram_tensor("w1d", [NF, 128, NH, 128], F32R, kind="ExternalInput").ap()
    w2d = nc.dram_tensor("w2d", [NH, 2, 128, NFH, 128], F32R, kind="ExternalInput").ap()
    b1d = nc.dram_tensor("b1d", [NF, 128, 1], F32, kind="ExternalInput").ap()
    if TC:
        xTt = nc.dram_tensor("xTt", [128, NH, TC], BF16, kind="ExternalInput").ap()
        w1b = nc.dram_tensor("w1b", [NF, 128, NH, 128], BF16, kind="ExternalInput").ap()
        w2b = nc.dram_tensor("w2b", [NH, 2, 128, NFH, 128], BF16, kind="ExternalInput").ap()
    yT0 = nc.dram_tensor("yT0", [NH, 128, C], F32, kind="ExternalOutput").ap()
    yT1 = nc.dram_tensor("yT1", [NH, 128, C], F32, kind="ExternalOutput").ap()

    chunks = [(o, 512) for o in range(0, CM, 512)]

    with tile.TileContext(nc) as tc, ExitStack() as ctx:
        xp = ctx.enter_context(tc.tile_pool(name="x", bufs=1))
        bp = ctx.enter_context(tc.tile_pool(name="b1", bufs=1))
        w1p = ctx.enter_context(tc.tile_pool(name="w1", bufs=8))
        w2p = ctx.enter_context(tc.tile_pool(name="w2", bufs=2))
        hp = ctx.enter_context(tc.tile_pool(name="h", bufs=1))
        op = ctx.enter_context(tc.tile_pool(name="ostage", bufs=2))
        psp = ctx.enter_context(tc.tile_pool(name="ps", bufs=6, space="PSUM"))

        # PE warmup: cheap bf16 dummy matmuls on zeroed tiles keep the PE busy
        # while the first token/weight DMAs land, so HAM un-throttles early.
        wz = w1p.tile([128, 128], BF16, tag="warmw")
        xz = xp.tile([128, 512], BF16, tag="warmx")
        nc.vector.memset(wz[:], 0.0)
        nc.vector.memset(xz[:], 0.0)
        psw = psp.tile([128, 512], F32, tag="ps")
        for i in range(18):
            nc.tensor.matmul(psw[:], wz[:], xz[:], start=True, stop=True)

        # Tokens (transposed), chunk-0-first so fc1 can start early. Token/
        # bias/output DMAs ride the scalar HWDGE queue; weights ride sync's.
        xt = xp.tile([128, NH, CM], F32R)
        b1t = bp.tile([128, NF, 1], F32)
        if TC:
            xtt = xp.tile([128, NH, TC], BF16, tag="xtail")
        for ci, (o, ln) in enumerate(chunks):
            nc.scalar.dma_start(xt[:, :, o:o + ln], xT[:, :, o:o + ln])
            if ci == 0:
                nc.scalar.dma_start(b1t[:], b1d)
                if TC:
                    nc.scalar.dma_start(xtt[:], xTt[:])

        GRP = 4  # fc1 f-blocks processed chunk-major per group
        for half in range(2):
            ht = hp.tile([128, NFH, CM], F32R, tag="ht")
            if TC:
                htt = hp.tile([128, NFH, TC], BF16, tag="httail")

            def _fc1_tail(g0, w1bs):
                for fi, fb in enumerate(range(g0, g0 + GRP)):
                    ps = psp.tile([128, 512], F32, tag="ps")
                    for hb in range(NH):
                        nc.tensor.matmul(
                            ps[:, :TC],
                            w1bs[fi][:, hb, :],
                            xtt[:, hb, :],
                            start=(hb == 0),
                            stop=(hb == NH - 1),
                        )
                    nc.scalar.activation(
                        htt[:, fb, :],
                        ps[:, :TC],
                        mybir.ActivationFunctionType.Relu,
                        bias=b1t[:, half * NFH + fb, :],
                    )

            # --- fc1: hidden[fb] = relu(w1[:, fb].T @ x.T + b1[fb]) ---
            # bf16 tail compute for group k is deferred until after group
            # k+1's mains, so its weight DMA never stalls the PE.
            pending = []  # (g0, w1bs) awaiting tail compute
            for g0 in range(0, NFH, GRP):
                w1ts = []
                for fb in range(g0, g0 + GRP):
                    w1t = w1p.tile([128, NH, 128], F32R, tag="w1t")
                    nc.sync.dma_start(w1t[:], w1d[half * NFH + fb])
                    w1ts.append(w1t)
                if TC and g0 > 0:  # fetch PREVIOUS group's bf16 weights now
                    p0 = g0 - GRP
                    w1bs = []
                    for fb in range(p0, p0 + GRP):
                        w1bt = w1p.tile([128, NH, 128], BF16, tag="w1bt")
                        nc.sync.dma_start(w1bt[:], w1b[half * NFH + fb])
                        w1bs.append(w1bt)
                    pending.append((p0, w1bs))
                for (o, ln) in chunks:  # chunk-major inside the group
                    for fi, fb in enumerate(range(g0, g0 + GRP)):
                        ps = psp.tile([128, 512], F32, tag="ps")
                        for hb in range(NH):
                            nc.tensor.matmul(
                                ps[:, :ln],
                                w1ts[fi][:, hb, :],
                                xt[:, hb, o:o + ln],
                                start=(hb == 0),
                                stop=(hb == NH - 1),
                            )
                        nc.scalar.activation(
                            ht[:, fb, o:o + ln],
                            ps[:, :ln],
                            mybir.ActivationFunctionType.Relu,
                            bias=b1t[:, half * NFH + fb, :],
                        )
                if TC and pending:
                    _fc1_tail(*pending.pop(0))
            if TC:  # last group's bf16 weights + tail
                p0 = NFH - GRP
                w1bs = []
                for fb in range(p0, p0 + GRP):
                    w1bt = w1p.tile([128, NH, 128], BF16, tag="w1bt")
                    nc.sync.dma_start(w1bt[:], w1b[half * NFH + fb])
                    w1bs.append(w1bt)
                _fc1_tail(p0, w1bs)
            # --- fc2: yT{half}[hb] = w2[half, :, hb].T @ hidden ---
            yT = yT0 if half == 0 else yT1
            for hb in range(NH):
                w2t = w2p.tile([128, NFH, 128], F32R, tag="w2t")
                nc.sync.dma_start(w2t[:], w2d[hb, half])
                if TC:
                    w2bt = w2p.tile([128, NFH, 128], BF16, tag="w2bt")
                    nc.sync.dma_start(w2bt[:], w2b[hb, half])
                last = (half == 1 and hb == NH - 1)
                ot = op.tile([128, C], F32, tag="ot")
                if TC:  # tail first so the main chunks finish last cheaply
                    ps2 = psp.tile([128, 512], F32, tag="ps")
                    for fb in range(NFH):
                        nc.tensor.matmul(
                            ps2[:, :TC],
                            w2bt[:, fb, :],
                            htt[:, fb, :],
                            start=(fb == 0),
                            stop=(fb == NFH - 1),
                        )
                    nc.vector.tensor_copy(ot[:, CM:], ps2[:, :TC])
                for (o, ln) in chunks:
                    ps2 = psp.tile([128, 512], F32, tag="ps")
                    for fb in range(NFH):
                        nc.tensor.matmul(
                            ps2[:, :ln],
                            w2t[:, fb, :],
                            ht[:, fb, o:o + ln],
                            start=(fb == 0),
                            stop=(fb == NFH - 1),
                        )
                    nc.vector.tensor_copy(ot[:, o:o + ln], ps2[:, :ln])
                    if last:
                        nc.scalar.dma_start(yT[hb, :, o:o + ln], ot[:, o:o + ln])
                if last:
                    if TC:
                        nc.scalar.dma_start(yT[hb, :, CM:], ot[:, CM:])
                else:
                    nc.scalar.dma_start(yT[hb], ot[:])

    nc.compile()
    return nc


def _get_program(CM, TC):
    if (CM, TC) not in _PROG_CACHE:
        _PROG_CACHE[(CM, TC)] = _build_program(CM, TC)
    return _PROG_CACHE[(CM, TC)]


def kernel(hidden_states, gate_w, w1, b1, w2, b2):
    x = np.ascontiguousarray(np.asarray(hidden_states, np.float32).reshape(T, H))
    gw = np.asarray(gate_w, np.float32)
    w1 = np.asarray(w1, np.float32)
    b1 = np.asarray(b1, np.float32)
    w2 = np.asarray(w2, np.float32)
    b2 = np.asarray(b2, np.float32)

    # --- gate (host, replicates reference math) ---
    logits = (x @ gw).astype(np.float32)                   # [T, E]
    lm = logits.max(-1, keepdims=True)
    p = np.exp(logits - lm, dtype=np.float32)
    p = p / p.sum(-1, keepdims=True, dtype=np.float32)     # softmax [T, E]
    order = np.argsort(-p, axis=-1, kind="stable")
    topk_idx = order[:, :K]                                # [T, K]
    topk_p = np.take_along_axis(p, topk_idx, axis=-1)
    topk_p = topk_p / topk_p.sum(-1, keepdims=True)
    pm = p.mean(axis=0, dtype=np.float64)
    aux_loss = np.float32(E * np.sum(pm * pm))

    # Per-expert token lists + combine weights, largest weight first so that
    # any bf16 tail slots get the least-weighted tokens.
    idx_lists, cw_lists = [], []
    for e in range(E):
        rows, cols = np.nonzero(topk_idx == e)
        cw = topk_p[rows, cols].astype(np.float32)
        o = np.argsort(-cw, kind="stable")
        idx_lists.append(rows[o].astype(np.int64))
        cw_lists.append(cw[o])
    max_n = max(len(i) for i in idx_lists)

    CM = max(512, (max_n // 512) * 512)                    # f32r main capacity
    rem = max_n - CM
    TC = ((rem + 127) // 128) * 128 if rem > 0 else 0      # bf16 tail capacity
    C = CM + TC

    nc = _get_program(CM, TC)

    in_maps = []
    for e in range(E):
        idx = idx_lists[e]
        pad = np.zeros(C, np.int64)
        pad[: len(idx)] = idx
        xg = x[pad]                                        # [C, H]
        xgT = np.ascontiguousarray(xg.T)                   # [H, C]
        w1t = np.ascontiguousarray(
            w1[e].reshape(NH, 128, NF, 128).transpose(2, 1, 0, 3)
        )                                                  # [NF,128,NH,128]
        w2t = np.ascontiguousarray(
            w2[e].reshape(2, NFH, 128, NH, 128).transpose(3, 0, 2, 1, 4)
        )                                                  # [NH,2,128,NFH,128]
        m = {
            "xT": np.ascontiguousarray(
                xgT[:, :CM].reshape(NH, 128, CM).transpose(1, 0, 2)
            ),
            "w1d": w1t,
            "w2d": w2t,
            "b1d": np.ascontiguousarray(b1[e].reshape(NF, 128, 1)),
        }
        if TC:
            m["xTt"] = np.ascontiguousarray(
                xgT[:, CM:].astype(BF16_NP).reshape(NH, 128, TC).transpose(1, 0, 2)
            )
            m["w1b"] = w1t.astype(BF16_NP)
            m["w2b"] = w2t.astype(BF16_NP)
        in_maps.append(m)

    res = run_bass_kernel_spmd(nc, in_maps, list(range(E)))
    kernel.last_results = res

    out = np.zeros((T, H), np.float32)
    for e in range(E):
        n = len(idx_lists[e])
        yt = res.results[e]["yT0"] + res.results[e]["yT1"]
        y = yt.transpose(2, 0, 1).reshape(C, H)            # [C, H]
        out[idx_lists[e]] += cw_lists[e][:, None] * y[:n]
    # b2 term: reference adds b2 inside each expert then weights by combine;
    # equivalent to adding sum_e c[t,e]*b2[e] here.
    cdense = np.zeros((T, E), np.float32)
    np.put_along_axis(cdense, topk_idx, topk_p, axis=-1)
    out += cdense @ b2
    return out.reshape(B, S, H), aux_loss


# revision 22
# speedup vs baseline: 1.1011x; 1.1011x over previous
"""Trainium2 Bass kernel for a top-2 MoE layer (B=2, S=2048, H=1024, F=4096, E=8).

Strategy: expert-parallel across 8 NeuronCores. The (tiny) gate runs on host in
numpy; each core runs one expert's fc1->relu->fc2 over only the tokens routed to
it, padded to a fixed capacity. The first C_main tokens (largest combine
weights) compute in float32r (full-rate PE matmul, ~1e-4 error); the tail
(capacity remainder, smallest combine weights) computes in bf16 — bf16 streams
at 1 cyc/col at any width, so the tail can be 128-granular instead of 256.
Host scatter-adds the combine-weighted expert outputs into the dense output.

Self-contained: hardcodes all shapes; imports only concourse + numpy.
"""

import numpy as np
import ml_dtypes
from contextlib import ExitStack

import concourse.bass as bass  # noqa: F401
import concourse.tile as tile
from concourse import bacc, mybir
from concourse.bass_utils import run_bass_kernel_spmd

B, S, H, F, E, K = 2, 2048, 1024, 4096, 8, 2
T = B * S
NH = H // 128   # 8 partition blocks of H
NF = F // 128   # 32 partition blocks of F
NFH = NF // 2   # f-blocks per F-half
F32 = mybir.dt.float32
F32R = mybir.dt.float32r
BF16 = mybir.dt.bfloat16
BF16_NP = ml_dtypes.bfloat16

_PROG_CACHE: dict = {}


def _build_program(CM, TC):
    """One expert's MLP over CM f32r tokens + TC bf16 tail tokens. Weights are
    streamed in a single pass by splitting F into two halves; the two halves'
    fc2 outputs go to separate DRAM tensors (host adds them)."""
    C = CM + TC
    nc = bacc.Bacc("TRN2", target_bir_lowering=False, debug=False, num_devices=E)

    xT = nc.dram_tensor("xT", [128, NH, CM], F32R, kind="ExternalInput").ap()
    w1d = nc.dram_tensor("w1d", [NF, 128, NH, 128], F32R, kind="ExternalInput").ap()
    w2d = nc.dram_tensor("w2d", [NH, 2, 128, NFH, 128], F32R, kind="ExternalInput").ap()
    b1d = nc.dram_tensor("b1d", [NF, 128, 1], F32, kind="ExternalInput").ap()
    if TC:
        xTt = nc.dram_tensor("xTt", [128, NH, TC], BF16, kind="ExternalInput").ap()
        w1b = nc.dram_tensor("w1b", [NF, 128, NH, 128], BF16, kind="ExternalInput").ap()
        w2b = nc.dram_tensor("w2b", [NH, 2, 128, NFH, 128], BF16, kind="ExternalInput").ap()
    yT0 = nc.dram_tensor("yT0", [NH, 128, C], F32, kind="ExternalOutput").ap()
    yT1 = nc.dram_tensor("yT1", [NH, 128, C], F32, kind="ExternalOutput").ap()

    chunks = [(o, 512) for o in range(0, CM, 512)]

    with tile.TileContext(nc) as tc, ExitStack() as ctx:
        xp = ctx.enter_context(tc.tile_pool(name="x", bufs=1))
        bp = ctx.enter_context(tc.tile_pool(name="b1", bufs=1))
        w1p = ctx.enter_context(tc.tile_pool(name="w1", bufs=8))
        w2p = ctx.enter_context(tc.tile_pool(name="w2", bufs=2))
        hp = ctx.enter_context(tc.tile_pool(name="h", bufs=1))
        op = ctx.enter_context(tc.tile_pool(name="ostage", bufs=2))
        psp = ctx.enter_context(tc.tile_pool(name="ps", bufs=6, space="PSUM"))

        # PE warmup: cheap bf16 dummy matmuls on zeroed tiles keep the PE busy
        # while the first token/weight DMAs land, so HAM un-throttles early.
        wz = w1p.tile([128, 128], BF16, tag="warmw")
        xz = xp.tile([128, 512], BF16, tag="warmx")
        nc.vector.memset(wz[:], 0.0)
        nc.vector.memset(xz[:], 0.0)
        psw = psp.tile([128, 512], F32, tag="ps")
        for i in range(18):
            nc.tensor.matmul(psw[:], wz[:], xz[:], start=True, stop=True)

        # Tokens (transposed), chunk-0-first so fc1 can start early. Token/
        # bias/output DMAs ride the scalar HWDGE queue; weights ride sync's.
        xt = xp.tile([128, NH, CM], F32R)
        b1t = bp.tile([128, NF, 1], F32)
        if TC:
            xtt = xp.tile([128, NH, TC], BF16, tag="xtail")
        for ci, (o, ln) in enumerate(chunks):
            nc.scalar.dma_start(xt[:, :, o:o + ln], xT[:, :, o:o + ln])
            if ci == 0:
                nc.scalar.dma_start(b1t[:], b1d)
                if TC:
                    nc.scalar.dma_start(xtt[:], xTt[:])

        GRP = 4  # fc1 f-blocks processed chunk-major per group
        for half in range(2):
            ht = hp.tile([128, NFH, CM], F32R, tag="ht")
            if TC:
                htt = hp.tile([128, NFH, TC], BF16, tag="httail")

            def _fc1_tail(g0, w1bs):
                for fi, fb in enumerate(range(g0, g0 + GRP)):
                    ps = psp.tile([128, 512], F32, tag="ps")
                    for hb in range(NH):
                        nc.tensor.matmul(
                            ps[:, :TC],
                            w1bs[fi][:, hb, :],
                            xtt[:, hb, :],
                            start=(hb == 0),
                            stop=(hb == NH - 1),
                        )
                    nc.scalar.activation(
                        htt[:, fb, :],
                        ps[:, :TC],
                        mybir.ActivationFunctionType.Relu,
                        bias=b1t[:, half * NFH + fb, :],
                    )

            # --- fc1: hidden[fb] = relu(w1[:, fb].T @ x.T + b1[fb]) ---
            # bf16 tail compute for group k is deferred until after group
            # k+1's mains, so its weight DMA never stalls the PE.
            pending = []  # (g0, w1bs) awaiting tail compute
            for g0 in range(0, NFH, GRP):
                w1ts = []
                for fb in range(g0, g0 + GRP):
                    w1t = w1p.tile([128, NH, 128], F32R, tag="w1t")
                    nc.sync.dma_start(w1t[:], w1d[half * NFH + fb])
                    w1ts.append(w1t)
                if TC and g0 > 0:  # fetch PREVIOUS group's bf16 weights now
                    p0 = g0 - GRP
                    w1bs = []
                    for fb in range(p0, p0 + GRP):
                        w1bt = w1p.tile([128, NH, 128], BF16, tag="w1bt")
                        nc.sync.dma_start(w1bt[:], w1b[half * NFH + fb])
                        w1bs.append(w1bt)
                    pending.append((p0, w1bs))
                for (o, ln) in chunks:  # chunk-major inside the group
                    for fi, fb in enumerate(range(g0, g0 + GRP)):
                        ps = psp.tile([128, 512], F32, tag="ps")
                        for hb in range(NH):
                            nc.tensor.matmul(
                                ps[:, :ln],
                                w1ts[fi][:, hb, :],
                                xt[:, hb, o:o + ln],
                                start=(hb == 0),
                                stop=(hb == NH - 1),
                            )
                        nc.scalar.activation(
                            ht[:, fb, o:o + ln],
                            ps[:, :ln],
                            mybir.ActivationFunctionType.Relu,
                            bias=b1t[:, half * NFH + fb, :],
                        )
                if TC and pending:
                    _fc1_tail(*pending.pop(0))
            if TC:  # last group's bf16 weights + tail
                p0 = NFH - GRP
                w1bs = []
                for fb in range(p0, p0 + GRP):
                    w1bt = w1p.tile([128, NH, 128], BF16, tag="w1bt")
                    nc.sync.dma_start(w1bt[:], w1b[half * NFH + fb])
                    w1bs.append(w1bt)
                _fc1_tail(p0, w1bs)
            # --- fc2: yT{half}[hb] = w2[half, :, hb].T @ hidden ---
            yT = yT0 if half == 0 else yT1
            for hb in range(NH):
                w2t = w2p.tile([128, NFH, 128], F32R, tag="w2t")
                nc.sync.dma_start(w2t[:], w2d[hb, half])
                if TC:
                    w2bt = w2p.tile([128, NFH, 128], BF16, tag="w2bt")
                    nc.sync.dma_start(w2bt[:], w2b[hb, half])
                last = (half == 1 and hb == NH - 1)
                ot = op.tile([128, C], F32, tag="ot")
                if TC:  # tail first so the main chunks finish last cheaply
                    ps2 = psp.tile([128, 512], F32, tag="ps")
                    for fb in range(NFH):
                        nc.tensor.matmul(
                            ps2[:, :TC],
                            w2bt[:, fb, :],
                            htt[:, fb, :],
                            start=(fb == 0),
                            stop=(fb == NFH - 1),
                        )
                    nc.vector.tensor_copy(ot[:, CM:], ps2[:, :TC])
                for (o, ln) in chunks:
                    ps2 = psp.tile([128, 512], F32, tag="ps")
                    for fb in range(NFH):
                        nc.tensor.matmul(
                            ps2[:, :ln],
                            w2t[:, fb, :],
                            ht[:, fb, o:o + ln],
                            start=(fb == 0),
                            stop=(fb == NFH - 1),
                        )
                    nc.vector.tensor_copy(ot[:, o:o + ln], ps2[:, :ln])
                    if last:
                        nc.scalar.dma_start(yT[hb, :, o:o + ln], ot[:, o:o + ln])
                if last:
                    if TC:
                        nc.scalar.dma_start(yT[hb, :, CM:], ot[:, CM:])
                else:
                    nc.scalar.dma_start(yT[hb], ot[:])

    nc.compile()
    return nc


def _get_program(CM, TC):
    if (CM, TC) not in _PROG_CACHE:
        _PROG_CACHE[(CM, TC)] = _build_program(CM, TC)
    return _PROG_CACHE[(CM, TC)]


def kernel(hidden_states, gate_w, w1, b1, w2, b2):
    x = np.ascontiguousarray(np.asarray(hidden_states, np.float32).reshape(T, H))
    gw = np.asarray(gate_w, np.float32)
    w1 = np.asarray(w1, np.float32)
    b1 = np.asarray(b1, np.float32)
    w2 = np.asarray(w2, np.float32)
    b2 = np.asarray(b2, np.float32)

    # --- gate (host, replicates reference math) ---
    logits = (x @ gw).astype(np.float32)                   # [T, E]
    lm = logits.max(-1, keepdims=True)
    p = np.exp(logits - lm, dtype=np.float32)
    p = p / p.sum(-1, keepdims=True, dtype=np.float32)     # softmax [T, E]
    order = np.argsort(-p, axis=-1, kind="stable")
    topk_idx = order[:, :K]                                # [T, K]
    topk_p = np.take_along_axis(p, topk_idx, axis=-1)
    topk_p = topk_p / topk_p.sum(-1, keepdims=True)
    pm = p.mean(axis=0, dtype=np.float64)
    aux_loss = np.float32(E * np.sum(pm * pm))

    # Per-expert token lists + combine weights, largest weight first so that
    # any bf16 tail slots get the least-weighted tokens.
    idx_lists, cw_lists = [], []
    for e in range(E):
        rows, cols = np.nonzero(topk_idx == e)
        cw = topk_p[rows, cols].astype(np.float32)
        o = np.argsort(-cw, kind="stable")
        idx_lists.append(rows[o].astype(np.int64))
        cw_lists.append(cw[o])
    max_n = max(len(i) for i in idx_lists)

    CM = max(512, (max_n // 512) * 512)                    # f32r main capacity
    rem = max_n - CM
    # Small overflow is computed exactly on host (a handful of token pairs);
    # large overflow gets a bf16 device tail (128-granular).
    TC = ((rem + 127) // 128) * 128 if rem > 384 else 0
    C = CM + TC

    nc = _get_program(CM, TC)

    in_maps = []
    for e in range(E):
        idx = idx_lists[e][:C]
        pad = np.zeros(C, np.int64)
        pad[: len(idx)] = idx
        xg = x[pad]                                        # [C, H]
        xgT = np.ascontiguousarray(xg.T)                   # [H, C]
        w1t = np.ascontiguousarray(
            w1[e].reshape(NH, 128, NF, 128).transpose(2, 1, 0, 3)
        )                                                  # [NF,128,NH,128]
        w2t = np.ascontiguousarray(
            w2[e].reshape(2, NFH, 128, NH, 128).transpose(3, 0, 2, 1, 4)
        )                                                  # [NH,2,128,NFH,128]
        m = {
            "xT": np.ascontiguousarray(
                xgT[:, :CM].reshape(NH, 128, CM).transpose(1, 0, 2)
            ),
            "w1d": w1t,
            "w2d": w2t,
            "b1d": np.ascontiguousarray(b1[e].reshape(NF, 128, 1)),
        }
        if TC:
            m["xTt"] = np.ascontiguousarray(
                xgT[:, CM:].astype(BF16_NP).reshape(NH, 128, TC).transpose(1, 0, 2)
            )
            m["w1b"] = w1t.astype(BF16_NP)
            m["w2b"] = w2t.astype(BF16_NP)
        in_maps.append(m)

    res = run_bass_kernel_spmd(nc, in_maps, list(range(E)))
    kernel.last_results = res

    out = np.zeros((T, H), np.float32)
    for e in range(E):
        n = len(idx_lists[e])
        nd = min(n, C)
        yt = res.results[e]["yT0"] + res.results[e]["yT1"]
        y = yt.transpose(2, 0, 1).reshape(C, H)            # [C, H]
        out[idx_lists[e][:nd]] += cw_lists[e][:nd, None] * y[:nd]
        if n > nd:  # overflow token pairs: exact host compute
            rows = idx_lists[e][nd:]
            yo = np.maximum(x[rows] @ w1[e] + b1[e], 0.0) @ w2[e]
            out[rows] += cw_lists[e][nd:, None] * yo
    # b2 term: reference adds b2 inside each expert then weights by combine;
    # equivalent to adding sum_e c[t,e]*b2[e] here.
    cdense = np.zeros((T, E), np.float32)
    np.put_along_axis(cdense, topk_idx, topk_p, axis=-1)
    out += cdense @ b2
    return out.reshape(B, S, H), aux_loss


# revision 23
# speedup vs baseline: 1.1056x; 1.0041x over previous
"""Trainium2 Bass kernel for a top-2 MoE layer (B=2, S=2048, H=1024, F=4096, E=8).

Strategy: expert-parallel across 8 NeuronCores. The (tiny) gate runs on host in
numpy; each core runs one expert's fc1->relu->fc2 over only the tokens routed to
it, padded to a fixed capacity. The first C_main tokens (largest combine
weights) compute in float32r (full-rate PE matmul, ~1e-4 error); the tail
(capacity remainder, smallest combine weights) computes in bf16 — bf16 streams
at 1 cyc/col at any width, so the tail can be 128-granular instead of 256.
Host scatter-adds the combine-weighted expert outputs into the dense output.

Self-contained: hardcodes all shapes; imports only concourse + numpy.
"""

import numpy as np
import ml_dtypes
from contextlib import ExitStack

import concourse.bass as bass  # noqa: F401
import concourse.tile as tile
from concourse import bacc, mybir
from concourse.bass_utils import run_bass_kernel_spmd

B, S, H, F, E, K = 2, 2048, 1024, 4096, 8, 2
T = B * S
NH = H // 128   # 8 partition blocks of H
NF = F // 128   # 32 partition blocks of F
NFH = NF // 2   # f-blocks per F-half
F32 = mybir.dt.float32
F32R = mybir.dt.float32r
BF16 = mybir.dt.bfloat16
BF16_NP = ml_dtypes.bfloat16

_PROG_CACHE: dict = {}


def _build_program(CM, TC):
    """One expert's MLP over CM f32r tokens + TC bf16 tail tokens. Weights are
    streamed in a single pass by splitting F into two halves; the two halves'
    fc2 outputs go to separate DRAM tensors (host adds them)."""
    C = CM + TC
    nc = bacc.Bacc("TRN2", target_bir_lowering=False, debug=False, num_devices=E)

    xT = nc.dram_tensor("xT", [128, NH, CM], F32R, kind="ExternalInput").ap()
    w1d = nc.dram_tensor("w1d", [NF, 128, NH, 128], F32R, kind="ExternalInput").ap()
    w2d = nc.dram_tensor("w2d", [NH, 2, 128, NFH, 128], F32R, kind="ExternalInput").ap()
    b1d = nc.dram_tensor("b1d", [NF, 128, 1], F32, kind="ExternalInput").ap()
    if TC:
        xTt = nc.dram_tensor("xTt", [128, NH, TC], BF16, kind="ExternalInput").ap()
        w1b = nc.dram_tensor("w1b", [NF, 128, NH, 128], BF16, kind="ExternalInput").ap()
        w2b = nc.dram_tensor("w2b", [NH, 2, 128, NFH, 128], BF16, kind="ExternalInput").ap()
    yT0 = nc.dram_tensor("yT0", [NH, 128, C], F32, kind="ExternalOutput").ap()
    yT1 = nc.dram_tensor("yT1", [NH, 128, C], F32, kind="ExternalOutput").ap()

    chunks = [(o, 512) for o in range(0, CM, 512)]

    with tile.TileContext(nc) as tc, ExitStack() as ctx:
        xp = ctx.enter_context(tc.tile_pool(name="x", bufs=1))
        bp = ctx.enter_context(tc.tile_pool(name="b1", bufs=1))
        w1p = ctx.enter_context(tc.tile_pool(name="w1", bufs=8))
        w2p = ctx.enter_context(tc.tile_pool(name="w2", bufs=4))
        hp = ctx.enter_context(tc.tile_pool(name="h", bufs=1))
        op = ctx.enter_context(tc.tile_pool(name="ostage", bufs=3))
        psp = ctx.enter_context(tc.tile_pool(name="ps", bufs=6, space="PSUM"))

        # PE warmup: cheap bf16 dummy matmuls on zeroed tiles keep the PE busy
        # while the first token/weight DMAs land, so HAM un-throttles early.
        wz = w1p.tile([128, 128], BF16, tag="warmw")
        xz = xp.tile([128, 512], BF16, tag="warmx")
        nc.vector.memset(wz[:], 0.0)
        nc.vector.memset(xz[:], 0.0)
        psw = psp.tile([128, 512], F32, tag="ps")
        for i in range(22):
            nc.tensor.matmul(psw[:], wz[:], xz[:], start=True, stop=True)

        # Tokens (transposed), chunk-0-first so fc1 can start early. Token/
        # bias/output DMAs ride the scalar HWDGE queue; weights ride sync's.
        xt = xp.tile([128, NH, CM], F32R)
        b1t = bp.tile([128, NF, 1], F32)
        if TC:
            xtt = xp.tile([128, NH, TC], BF16, tag="xtail")
        # Token DMA pieces: a split first chunk lets fc1 start after only
        # 1MB of tokens has landed.
        tok_pieces = [(0, 256), (256, 256)] + [(o, 512) for o in range(512, CM, 512)]
        for ci, (o, ln) in enumerate(tok_pieces):
            nc.scalar.dma_start(xt[:, :, o:o + ln], xT[:, :, o:o + ln])
            if ci == 0:
                nc.scalar.dma_start(b1t[:], b1d)
                if TC:
                    nc.scalar.dma_start(xtt[:], xTt[:])

        # fc1 f-block group sizes: small leading groups so the PE can start
        # after only 1MB of w1 has landed.
        gsizes = [2, 2] + [4] * ((NFH - 4) // 4)
        groups = []
        f0 = 0
        for gs in gsizes:
            groups.append(list(range(f0, f0 + gs)))
            f0 += gs

        for half in range(2):
            ht = hp.tile([128, NFH, CM], F32R, tag="ht")
            if TC:
                htt = hp.tile([128, NFH, TC], BF16, tag="httail")

            def _fc1_tail(fbs, w1bs):
                for fi, fb in enumerate(fbs):
                    ps = psp.tile([128, 512], F32, tag="ps")
                    for hb in range(NH):
                        nc.tensor.matmul(
                            ps[:, :TC],
                            w1bs[fi][:, hb, :],
                            xtt[:, hb, :],
                            start=(hb == 0),
                            stop=(hb == NH - 1),
                        )
                    nc.scalar.activation(
                        htt[:, fb, :],
                        ps[:, :TC],
                        mybir.ActivationFunctionType.Relu,
                        bias=b1t[:, half * NFH + fb, :],
                    )

            # --- fc1: hidden[fb] = relu(w1[:, fb].T @ x.T + b1[fb]) ---
            # bf16 tail compute for group k is deferred until after group
            # k+1's mains, so its weight DMA never stalls the PE.
            pending = []  # (fbs, w1bs) awaiting tail compute
            for gi, fbs in enumerate(groups):
                w1ts = []
                for fb in fbs:
                    w1t = w1p.tile([128, NH, 128], F32R, tag="w1t")
                    nc.sync.dma_start(w1t[:], w1d[half * NFH + fb])
                    w1ts.append(w1t)
                if TC and gi > 0:  # fetch PREVIOUS group's bf16 weights now
                    pfbs = groups[gi - 1]
                    w1bs = []
                    for fb in pfbs:
                        w1bt = w1p.tile([128, NH, 128], BF16, tag="w1bt")
                        nc.sync.dma_start(w1bt[:], w1b[half * NFH + fb])
                        w1bs.append(w1bt)
                    pending.append((pfbs, w1bs))
                gchunks = tok_pieces if gi == 0 else chunks
                for (o, ln) in gchunks:  # chunk-major inside the group
                    for fi, fb in enumerate(fbs):
                        ps = psp.tile([128, 512], F32, tag="ps")
                        for hb in range(NH):
                            nc.tensor.matmul(
                                ps[:, :ln],
                                w1ts[fi][:, hb, :],
                                xt[:, hb, o:o + ln],
                                start=(hb == 0),
                                stop=(hb == NH - 1),
                            )
                        nc.scalar.activation(
                            ht[:, fb, o:o + ln],
                            ps[:, :ln],
                            mybir.ActivationFunctionType.Relu,
                            bias=b1t[:, half * NFH + fb, :],
                        )
                if TC and pending:
                    _fc1_tail(*pending.pop(0))
            if TC:  # last group's bf16 weights + tail
                pfbs = groups[-1]
                w1bs = []
                for fb in pfbs:
                    w1bt = w1p.tile([128, NH, 128], BF16, tag="w1bt")
                    nc.sync.dma_start(w1bt[:], w1b[half * NFH + fb])
                    w1bs.append(w1bt)
                _fc1_tail(pfbs, w1bs)
            # --- fc2: yT{half}[hb] = w2[half, :, hb].T @ hidden ---
            yT = yT0 if half == 0 else yT1
            for hb in range(NH):
                w2t = w2p.tile([128, NFH, 128], F32R, tag="w2t")
                nc.sync.dma_start(w2t[:], w2d[hb, half])
                if TC:
                    w2bt = w2p.tile([128, NFH, 128], BF16, tag="w2bt")
                    nc.sync.dma_start(w2bt[:], w2b[hb, half])
                last = (half == 1 and hb == NH - 1)
                ot = op.tile([128, C], F32, tag="ot")
                if TC:  # tail first so the main chunks finish last cheaply
                    ps2 = psp.tile([128, 512], F32, tag="ps")
                    for fb in range(NFH):
                        nc.tensor.matmul(
                            ps2[:, :TC],
                            w2bt[:, fb, :],
                            htt[:, fb, :],
                            start=(fb == 0),
                            stop=(fb == NFH - 1),
                        )
                    nc.vector.tensor_copy(ot[:, CM:], ps2[:, :TC])
                for (o, ln) in chunks:
                    ps2 = psp.tile([128, 512], F32, tag="ps")
                    for fb in range(NFH):
                        nc.tensor.matmul(
                            ps2[:, :ln],
                            w2t[:, fb, :],
                            ht[:, fb, o:o + ln],
                            start=(fb == 0),
                            stop=(fb == NFH - 1),
                        )
                    nc.vector.tensor_copy(ot[:, o:o + ln], ps2[:, :ln])
                    if last:
                        nc.scalar.dma_start(yT[hb, :, o:o + ln], ot[:, o:o + ln])
                if last:
                    if TC:
                        nc.scalar.dma_start(yT[hb, :, CM:], ot[:, CM:])
                else:
                    nc.scalar.dma_start(yT[hb], ot[:])

    nc.compile()
    return nc


def _get_program(CM, TC):
    if (CM, TC) not in _PROG_CACHE:
        _PROG_CACHE[(CM, TC)] = _build_program(CM, TC)
    return _PROG_CACHE[(CM, TC)]


def kernel(hidden_states, gate_w, w1, b1, w2, b2):
    x = np.ascontiguousarray(np.asarray(hidden_states, np.float32).reshape(T, H))
    gw = np.asarray(gate_w, np.float32)
    w1 = np.asarray(w1, np.float32)
    b1 = np.asarray(b1, np.float32)
    w2 = np.asarray(w2, np.float32)
    b2 = np.asarray(b2, np.float32)

    # --- gate (host, replicates reference math) ---
    logits = (x @ gw).astype(np.float32)                   # [T, E]
    lm = logits.max(-1, keepdims=True)
    p = np.exp(logits - lm, dtype=np.float32)
    p = p / p.sum(-1, keepdims=True, dtype=np.float32)     # softmax [T, E]
    order = np.argsort(-p, axis=-1, kind="stable")
    topk_idx = order[:, :K]                                # [T, K]
    topk_p = np.take_along_axis(p, topk_idx, axis=-1)
    topk_p = topk_p / topk_p.sum(-1, keepdims=True)
    pm = p.mean(axis=0, dtype=np.float64)
    aux_loss = np.float32(E * np.sum(pm * pm))

    # Per-expert token lists + combine weights, largest weight first so that
    # any bf16 tail slots get the least-weighted tokens.
    idx_lists, cw_lists = [], []
    for e in range(E):
        rows, cols = np.nonzero(topk_idx == e)
        cw = topk_p[rows, cols].astype(np.float32)
        o = np.argsort(-cw, kind="stable")
        idx_lists.append(rows[o].astype(np.int64))
        cw_lists.append(cw[o])
    max_n = max(len(i) for i in idx_lists)

    CM = max(512, (max_n // 512) * 512)                    # f32r main capacity
    rem = max_n - CM
    # Small overflow is computed exactly on host (a handful of token pairs);
    # large overflow gets a bf16 device tail (128-granular).
    TC = ((rem + 127) // 128) * 128 if rem > 384 else 0
    C = CM + TC

    nc = _get_program(CM, TC)

    in_maps = []
    for e in range(E):
        idx = idx_lists[e][:C]
        pad = np.zeros(C, np.int64)
        pad[: len(idx)] = idx
        xg = x[pad]                                        # [C, H]
        xgT = np.ascontiguousarray(xg.T)                   # [H, C]
        w1t = np.ascontiguousarray(
            w1[e].reshape(NH, 128, NF, 128).transpose(2, 1, 0, 3)
        )                                                  # [NF,128,NH,128]
        w2t = np.ascontiguousarray(
            w2[e].reshape(2, NFH, 128, NH, 128).transpose(3, 0, 2, 1, 4)
        )                                                  # [NH,2,128,NFH,128]
        m = {
            "xT": np.ascontiguousarray(
                xgT[:, :CM].reshape(NH, 128, CM).transpose(1, 0, 2)
            ),
            "w1d": w1t,
            "w2d": w2t,
            "b1d": np.ascontiguousarray(b1[e].reshape(NF, 128, 1)),
        }
        if TC:
            m["xTt"] = np.ascontiguousarray(
                xgT[:, CM:].astype(BF16_NP).reshape(NH, 128, TC).transpose(1, 0, 2)
            )
            m["w1b"] = w1t.astype(BF16_NP)
            m["w2b"] = w2t.astype(BF16_NP)
        in_maps.append(m)

    res = run_bass_kernel_spmd(nc, in_maps, list(range(E)))
    kernel.last_results = res

    out = np.zeros((T, H), np.float32)
    for e in range(E):
        n = len(idx_lists[e])
        nd = min(n, C)
        yt = res.results[e]["yT0"] + res.results[e]["yT1"]
        y = yt.transpose(2, 0, 1).reshape(C, H)            # [C, H]
        out[idx_lists[e][:nd]] += cw_lists[e][:nd, None] * y[:nd]
        if n > nd:  # overflow token pairs: exact host compute
            rows = idx_lists[e][nd:]
            yo = np.maximum(x[rows] @ w1[e] + b1[e], 0.0) @ w2[e]
            out[rows] += cw_lists[e][nd:, None] * yo
    # b2 term: reference adds b2 inside each expert then weights by combine;
    # equivalent to adding sum_e c[t,e]*b2[e] here.
    cdense = np.zeros((T, E), np.float32)
    np.put_along_axis(cdense, topk_idx, topk_p, axis=-1)
    out += cdense @ b2
    return out.reshape(B, S, H), aux_loss
